# revision 7
# baseline (speedup 1.0000x reference)
"""Trainium2 Bass kernel for the LogPolarFoveatedConvSNN problem.

Data-parallel over batch: 8 cores, one batch item each. Host does the
(constant-grid) log-polar bilinear gather + im2col packing; the device runs
the T=16 recurrent conv/LIF pipeline; host does the cheap final reductions.
"""

import numpy as np
import ml_dtypes

# ---- problem constants (hardcoded per harness contract) ----
T, B, H, W, C = 16, 8, 224, 224, 3
RB, AB = 64, 128
C1, C2, OUT = 64, 128, 1000
BETA, THRESH = 0.9, 1.0
NCORES = 8
NBLK = 16           # spatial blocks per image (each 4 rows x 128 cols = 512)
PW = AB + 2         # padded width 130
PH = RB + 2         # padded height 66

# conv dtype config: "fp32" | "fp32r" for conv1; "bf16split" | "fp32r" for conv2
CFG = {"conv1": "fp32", "conv2": "fp32r"}
TRACE = False
TRACE_DIR = None

_compiled = {}
last_result_info = {}


# ---------------------------------------------------------------------------
# host-side log-polar sampling (mirrors reference._build_log_polar_grid in f32)
# ---------------------------------------------------------------------------
def _grid():
    cy, cx = (H - 1) / 2.0, (W - 1) / 2.0
    max_r = max(min(cy, cx) * 1.0, 1.0 + 0.001)
    radial_steps = np.linspace(0.0, 1.0, RB, dtype=np.float32)
    ang = np.arange(AB, dtype=np.float32) * np.float32(2.0 * np.pi / AB)
    log_min = np.float32(np.log(np.float32(1.0)))
    log_max = np.float32(np.log(np.float32(max_r)))
    radii = np.exp(log_min + radial_steps * (log_max - log_min)).astype(np.float32)
    ys = np.clip(cy + radii[:, None] * np.sin(ang)[None, :], 0.0, H - 1.0).astype(np.float32)
    xs = np.clip(cx + radii[:, None] * np.cos(ang)[None, :], 0.0, W - 1.0).astype(np.float32)
    return ys, xs


def _sample_log_polar(x_seq):
    ys, xs = _grid()
    y0 = np.floor(ys)
    x0 = np.floor(xs)
    wy = (ys - y0)[None, None, :, :, None].astype(np.float32)
    wx = (xs - x0)[None, None, :, :, None].astype(np.float32)
    y0i = np.clip(y0.astype(np.int32), 0, H - 1)
    x0i = np.clip(x0.astype(np.int32), 0, W - 1)
    y1i = np.clip(y0i + 1, 0, H - 1)
    x1i = np.clip(x0i + 1, 0, W - 1)
    g = lambda yy, xx: x_seq[:, :, yy, xx, :]  # [T,B,RB,AB,C]
    return (g(y0i, x0i) * (1 - wy) * (1 - wx)
            + g(y0i, x1i) * (1 - wy) * wx
            + g(y1i, x0i) * wy * (1 - wx)
            + g(y1i, x1i) * wy * wx).astype(np.float32)


def _pack_im2col(lp_seq):
    """lp_seq [T,B,RB,AB,C] -> per-core xcol [B][T, 128, 2048] f32.

    Row-tiled layout: spatial block b = 4q+g covers rows 16q+4g .. +3.
    partition = 32g + (3k + c) for tap k=3dy+dx, channel c.
    free = 512q + 128r + ab for in-block row r.
    """
    xpad = np.zeros((T, B, PH, PW, C), np.float32)
    xpad[:, :, 1:RB + 1, 1:AB + 1, :] = lp_seq
    x9 = np.stack([xpad[:, :, dy:dy + RB, dx:dx + AB, :]
                   for dy in range(3) for dx in range(3)])  # [9,T,B,RB,AB,C]
    # [k,t,B,(q g r),ab,c] -> [B,t,g,(k c),(q r ab)]
    x9 = x9.reshape(9, T, B, 4, 4, 4, AB, C)
    x9 = x9.transpose(2, 1, 4, 0, 7, 3, 5, 6)  # [B,t,g,k,c,q,r,ab]
    x9 = x9.reshape(B, T, 4, 27, 4 * 4 * AB)
    out = np.zeros((B, T, 4, 32, 2048), np.float32)
    out[:, :, :, :27, :] = x9
    return out.reshape(B, T, 128, 2048)


# ---------------------------------------------------------------------------
# device kernel build
# ---------------------------------------------------------------------------
def _build(cfg_key):
    import concourse.mybir as mybir
    import concourse.tile as tile
    from concourse import bacc

    fp32 = mybir.dt.float32
    fp32r = mybir.dt.float32r
    bf16 = mybir.dt.bfloat16
    Alu = mybir.AluOpType

    c1_dt = fp32 if CFG["conv1"] == "fp32" else fp32r
    c2_split = CFG["conv2"] == "bf16split"
    s1_dt = bf16 if c2_split else fp32r

    nc = bacc.Bacc(None, target_bir_lowering=False)

    xcol_d = nc.declare_dram_parameter("xcol", [T, 128, 2048], c1_dt, isOutput=False)
    wq_d = nc.declare_dram_parameter("wq", [128, C1], c1_dt, isOutput=False)
    if c2_split:
        w2h_d = nc.declare_dram_parameter("w2h", [C1, 9, C2], bf16, isOutput=False)
        w2l_d = nc.declare_dram_parameter("w2l", [C1, 9, C2], bf16, isOutput=False)
    else:
        w2h_d = nc.declare_dram_parameter("w2h", [C1, 9, C2], fp32r, isOutput=False)
    thrb1_d = nc.declare_dram_parameter("thrb1", [C1, 2], fp32, isOutput=False)
    thrb2_d = nc.declare_dram_parameter("thrb2", [C2, 2], fp32, isOutput=False)
    wh_d = nc.declare_dram_parameter("whm", [C2, OUT], fp32, isOutput=False)

    lgt_d = nc.declare_dram_parameter("lgt", [T, OUT], fp32, isOutput=True)
    s1s_d = nc.declare_dram_parameter("s1s", [C1, T], fp32, isOutput=True)
    s2s_d = nc.declare_dram_parameter("s2s", [C2, T], fp32, isOutput=True)

    with tile.TileContext(nc) as tc:
        with tc.tile_pool(name="consts", bufs=1) as consts, \
             tc.tile_pool(name="xin", bufs=2) as xin, \
             tc.tile_pool(name="work", bufs=4) as work, \
             tc.tile_pool(name="p1p", bufs=4, space="PSUM") as p1p, \
             tc.tile_pool(name="p2p", bufs=3, space="PSUM") as p2p, \
             tc.tile_pool(name="php", bufs=1, space="PSUM") as php:

            wq = consts.tile([128, C1], c1_dt)
            nc.sync.dma_start(wq, wq_d[:])
            if c2_split:
                w2h = consts.tile([C1, 9, C2], bf16)
                nc.sync.dma_start(w2h, w2h_d[:])
                w2l = consts.tile([C1, 9, C2], bf16)
                nc.sync.dma_start(w2l, w2l_d[:])
            else:
                w2h = consts.tile([C1, 9, C2], fp32r)
                nc.sync.dma_start(w2h, w2h_d[:])
                w2l = None
            thrb1 = consts.tile([C1, 2], fp32)
            nc.sync.dma_start(thrb1, thrb1_d[:])
            thrb2 = consts.tile([C2, 2], fp32)
            nc.sync.dma_start(thrb2, thrb2_d[:])
            whm = consts.tile([C2, OUT], fp32)
            nc.sync.dma_start(whm, wh_d[:])
            thr1, bia1 = thrb1[:, 0:1], thrb1[:, 1:2]
            thr2, bia2 = thrb2[:, 0:1], thrb2[:, 1:2]

            v1 = consts.tile([C1, RB * AB], fp32)
            nc.vector.memset(v1, 0.0)
            v2 = consts.tile([C2, RB * AB], fp32)
            nc.vector.memset(v2, 0.0)
            s1pad = [consts.tile([C1, PH, PW], s1_dt, name=f"s1pad{i}", tag=f"s1pad{i}")
                     for i in range(2)]
            for i in range(2):
                if s1_dt == fp32r:
                    nc.vector.memset(s1pad[i].bitcast(fp32), 0.0)
                else:
                    nc.vector.memset(s1pad[i], 0.0)
            s1bs = consts.tile([C1, T * NBLK], fp32)
            s2bs = consts.tile([C2, T * NBLK], fp32)
            s1sum = consts.tile([C1, T], fp32)
            s2sum = consts.tile([C2, T], fp32)

            def lif2_block(t, b, p2):
                bs = slice(b * 512, (b + 1) * 512)
                u2 = work.tile([C2, 512], fp32, tag="u2")
                nc.vector.scalar_tensor_tensor(
                    out=u2, in0=v2[:, bs], scalar=BETA, in1=p2,
                    op0=Alu.mult, op1=Alu.add)
                s2t = work.tile([C2, 512], bf16, tag="s2t")
                nc.vector.tensor_scalar(
                    out=s2t, in0=u2, scalar1=thr2, scalar2=0.0,
                    op0=Alu.is_gt, op1=Alu.add,
                    accum_out=s2bs[:, t * NBLK + b: t * NBLK + b + 1])
                nc.vector.scalar_tensor_tensor(
                    out=v2[:, bs], in0=u2, scalar=bia2, in1=s2t,
                    op0=Alu.add, op1=Alu.subtract)

            def conv2_block(t, b):
                sp = s1pad[t % 2]
                p2 = p2p.tile([C2, 512], fp32, tag="p2")
                n_mm = 18 if c2_split else 9
                i = 0
                for k in range(9):
                    dy, dx = k // 3, k % 3
                    rhs = sp[:, 4 * b + dy: 4 * b + dy + 4, dx: dx + AB]
                    nc.tensor.matmul(p2, w2h[:, k, :], rhs,
                                     start=(i == 0), stop=(i == n_mm - 1))
                    i += 1
                    if c2_split:
                        nc.tensor.matmul(p2, w2l[:, k, :], rhs,
                                         start=False, stop=(i == n_mm - 1))
                        i += 1
                lif2_block(t, b, p2)

            for t in range(T):
                xc = xin.tile([128, 2048], c1_dt, tag="xc")
                nc.sync.dma_start(xc, xcol_d[t])
                sp = s1pad[t % 2]
                for q in range(4):
                    ps = []
                    for g in range(4):
                        p1 = p1p.tile([C1, 512], fp32, tag="p1")
                        nc.tensor.matmul(
                            p1, wq[32 * g: 32 * g + 27, :],
                            xc[32 * g: 32 * g + 27, 512 * q: 512 * (q + 1)],
                            start=True, stop=True, tile_position=(32 * g, 0))
                        ps.append(p1)
                    for g in range(4):
                        b = 4 * q + g
                        bs = slice(b * 512, (b + 1) * 512)
                        u1 = work.tile([C1, 512], fp32, tag="u1")
                        nc.vector.scalar_tensor_tensor(
                            out=u1, in0=v1[:, bs], scalar=BETA, in1=ps[g],
                            op0=Alu.mult, op1=Alu.add)
                        spike_view = sp[:, 4 * b + 1: 4 * b + 5, 1: AB + 1]
                        nc.vector.tensor_scalar(
                            out=spike_view,
                            in0=u1.rearrange("p (a x) -> p a x", a=4),
                            scalar1=thr1, scalar2=0.0,
                            op0=Alu.is_gt, op1=Alu.add,
                            accum_out=s1bs[:, t * NBLK + b: t * NBLK + b + 1])
                        nc.vector.scalar_tensor_tensor(
                            out=v1[:, bs].rearrange("p (a x) -> p a x", a=4),
                            in0=u1.rearrange("p (a x) -> p a x", a=4),
                            scalar=bia1, in1=spike_view,
                            op0=Alu.add, op1=Alu.subtract)
                        # conv2 of block b-1 is now unblocked (needs spikes of b)
                        if b >= 1:
                            conv2_block(t, b - 1)
                conv2_block(t, NBLK - 1)

            # per-step channel sums
            nc.vector.reduce_sum(
                s1sum, s1bs.rearrange("p (t b) -> p t b", t=T),
                axis=mybir.AxisListType.X)
            nc.vector.reduce_sum(
                s2sum, s2bs.rearrange("p (t b) -> p t b", t=T),
                axis=mybir.AxisListType.X)
            nc.sync.dma_start(s1s_d[:], s1sum)
            nc.sync.dma_start(s2s_d[:], s2sum)

            # head: lgt[t, o] = sum_c s2sum[c, t] * wh[c, o]  (scaled on host)
            for half in range(2):
                ph = php.tile([T, 500], fp32, tag="ph")
                nc.tensor.matmul(ph, s2sum, whm[:, half * 500: (half + 1) * 500],
                                 start=True, stop=True)
                lo = work.tile([T, 500], fp32, tag="lo")
                nc.vector.tensor_copy(lo, ph)
                nc.sync.dma_start(lgt_d[:, half * 500: (half + 1) * 500], lo)

    nc.compile()
    return nc


def _get_nc():
    key = (CFG["conv1"], CFG["conv2"])
    if key not in _compiled:
        _compiled[key] = _build(key)
    return _compiled[key]


# ---------------------------------------------------------------------------
# entry point
# ---------------------------------------------------------------------------
def kernel(x_seq, w1, b1, w2, b2, wh, bh):
    from concourse.bass_utils import run_bass_kernel_spmd

    x_seq = np.asarray(x_seq, np.float32)
    w1 = np.asarray(w1, np.float32)
    b1 = np.asarray(b1, np.float32)
    w2 = np.asarray(w2, np.float32)
    b2 = np.asarray(b2, np.float32)
    wh = np.asarray(wh, np.float32)
    bh = np.asarray(bh, np.float32)

    lp_seq = _sample_log_polar(x_seq)              # [T,B,RB,AB,C]
    xcols = _pack_im2col(lp_seq)                   # [B,T,128,2048]

    w1col = w1.reshape(27, C1).astype(np.float32)
    wq = np.zeros((128, C1), np.float32)
    for g in range(4):
        wq[32 * g: 32 * g + 27] = w1col
    w2taps = np.ascontiguousarray(
        w2.reshape(9, C1, C2).transpose(1, 0, 2))  # [C1, 9, C2]
    thrb1 = np.stack([1.0 - b1, b1], axis=1).astype(np.float32)
    thrb2 = np.stack([1.0 - b2, b2], axis=1).astype(np.float32)

    base = dict(wq=wq, thrb1=thrb1, thrb2=thrb2, whm=wh)
    if CFG["conv2"] == "bf16split":
        w2h = w2taps.astype(ml_dtypes.bfloat16)
        w2l = (w2taps - w2h.astype(np.float32)).astype(ml_dtypes.bfloat16)
        base.update(w2h=w2h, w2l=w2l)
    else:
        base.update(w2h=w2taps)

    nc = _get_nc()
    in_maps = [dict(base, xcol=np.ascontiguousarray(xcols[i])) for i in range(B)]
    kw = {}
    if TRACE:
        kw = dict(trace=True, tmpdir=TRACE_DIR)
    res = run_bass_kernel_spmd(nc, in_maps, list(range(NCORES)), **kw)
    last_result_info.clear()
    last_result_info.update(exec_time_ns=res.exec_time_ns,
                            mean_exec_time_ns=res.mean_exec_time_ns)

    # ---- host-side assembly ----
    logits_seq = np.empty((T, B, OUT), np.float32)
    s1_counts = np.empty((B, C1, T), np.float32)
    s2_counts = np.empty((B, C2, T), np.float32)
    for i in range(B):
        r = res.results[i]
        logits_seq[:, i, :] = r["lgt"] * np.float32(1.0 / (RB * AB)) + bh
        s1_counts[i] = r["s1s"]
        s2_counts[i] = r["s2s"]

    readout = logits_seq.mean(axis=0)
    sr_seq = np.stack([
        s1_counts.sum(axis=(0, 1)) / np.float32(B * RB * AB * C1),
        s2_counts.sum(axis=(0, 1)) / np.float32(B * RB * AB * C2),
    ], axis=1).astype(np.float32)                  # [T, 2]
    sr = sr_seq.mean(axis=0).astype(np.float32)
    re_seq = lp_seq.mean(axis=(1, 3, 4))           # [T, RB]
    radial_energy = re_seq.mean(axis=0).astype(np.float32)

    return (readout, logits_seq, sr, lp_seq, radial_energy)


# revision 10
# speedup vs baseline: 1.4405x; 1.4405x over previous
"""Trainium2 Bass kernel for the LogPolarFoveatedConvSNN problem.

Data-parallel over batch: 8 cores, one batch item each. Host does the
(constant-grid) log-polar bilinear gather + im2col packing; the device runs
the T=16 recurrent conv/LIF pipeline; host does the cheap final reductions.

Device design (per core, per step):
  conv1: im2col K=27 matmuls, bf16 3-term split (xh*wh + xh*wl + xl*wh),
         4 spatial blocks concurrent via PE row-tiling.
  LIF1:  decay+add (DVE, PSUM src) -> spike is_gt written into a zero-padded
         [64,66,130] image (lower partitions of a 128-part tile) -> reset.
  conv2: 3x3 taps as matmuls accumulating in PSUM, fp32r (spikes exact),
         taps paired via a DMA-maintained column-shifted copy in upper
         partitions: 3 pair-matmuls (K=128) + 3 singles (K=64).
  LIF2:  decay+add (DVE) -> spike via Sign+Relu on ScalarE (with channel-sum
         accumulation) -> reset (DVE).
Head (once): logits = spike-count matrix @ wh on PE; scaling on host.
"""

import numpy as np
import ml_dtypes

# ---- problem constants (hardcoded per harness contract) ----
T, B, H, W, C = 16, 8, 224, 224, 3
RB, AB = 64, 128
C1, C2, OUT = 64, 128, 1000
BETA, THRESH = 0.9, 1.0
NCORES = 8
NBLK = 16           # spatial blocks per image (each 4 rows x 128 cols = 512)
PW = AB + 2         # padded width 130
PH = RB + 2         # padded height 66

TRACE = False
TRACE_DIR = None

_compiled = {}
last_result_info = {}

# tap index k = 3*dy + dx; flat padded offset = 130*dy + dx
PAIRS = [(0, 1), (4, 5), (6, 7)]     # delta=1 pairs (tap indices)
SINGLES = [2, 3, 8]                  # remaining taps


# ---------------------------------------------------------------------------
# host-side log-polar sampling (mirrors reference._build_log_polar_grid in f32)
# ---------------------------------------------------------------------------
def _grid():
    cy, cx = (H - 1) / 2.0, (W - 1) / 2.0
    max_r = max(min(cy, cx) * 1.0, 1.0 + 0.001)
    radial_steps = np.linspace(0.0, 1.0, RB, dtype=np.float32)
    ang = np.arange(AB, dtype=np.float32) * np.float32(2.0 * np.pi / AB)
    log_min = np.float32(np.log(np.float32(1.0)))
    log_max = np.float32(np.log(np.float32(max_r)))
    radii = np.exp(log_min + radial_steps * (log_max - log_min)).astype(np.float32)
    ys = np.clip(cy + radii[:, None] * np.sin(ang)[None, :], 0.0, H - 1.0).astype(np.float32)
    xs = np.clip(cx + radii[:, None] * np.cos(ang)[None, :], 0.0, W - 1.0).astype(np.float32)
    return ys, xs


def _sample_log_polar(x_seq):
    ys, xs = _grid()
    y0 = np.floor(ys)
    x0 = np.floor(xs)
    wy = (ys - y0)[None, None, :, :, None].astype(np.float32)
    wx = (xs - x0)[None, None, :, :, None].astype(np.float32)
    y0i = np.clip(y0.astype(np.int32), 0, H - 1)
    x0i = np.clip(x0.astype(np.int32), 0, W - 1)
    y1i = np.clip(y0i + 1, 0, H - 1)
    x1i = np.clip(x0i + 1, 0, W - 1)
    g = lambda yy, xx: x_seq[:, :, yy, xx, :]  # [T,B,RB,AB,C]
    return (g(y0i, x0i) * (1 - wy) * (1 - wx)
            + g(y0i, x1i) * (1 - wy) * wx
            + g(y1i, x0i) * wy * (1 - wx)
            + g(y1i, x1i) * wy * wx).astype(np.float32)


def _pack_im2col(lp_seq):
    """lp_seq [T,B,RB,AB,C] -> per-core xcol [B][T, 128, 2048] f32.

    Row-tiled layout: spatial block b = 4q+g covers rows 16q+4g .. +3.
    partition = 32g + (3k + c) for tap k=3dy+dx, channel c.
    free = 512q + 128r + ab for in-block row r.
    """
    xpad = np.zeros((T, B, PH, PW, C), np.float32)
    xpad[:, :, 1:RB + 1, 1:AB + 1, :] = lp_seq
    x9 = np.stack([xpad[:, :, dy:dy + RB, dx:dx + AB, :]
                   for dy in range(3) for dx in range(3)])  # [9,T,B,RB,AB,C]
    x9 = x9.reshape(9, T, B, 4, 4, 4, AB, C)
    x9 = x9.transpose(2, 1, 4, 0, 7, 3, 5, 6)  # [B,t,g,k,c,q,r,ab]
    x9 = x9.reshape(B, T, 4, 27, 4 * 4 * AB)
    out = np.zeros((B, T, 4, 32, 2048), np.float32)
    out[:, :, :, :27, :] = x9
    return out.reshape(B, T, 128, 2048)


# ---------------------------------------------------------------------------
# device kernel build
# ---------------------------------------------------------------------------
def _build(ZB):
    import concourse.mybir as mybir
    import concourse.tile as tile
    from concourse import bacc

    fp32 = mybir.dt.float32
    fp32r = mybir.dt.float32r
    bf16 = mybir.dt.bfloat16
    Alu = mybir.AluOpType
    Act = mybir.ActivationFunctionType

    nc = bacc.Bacc(None, target_bir_lowering=False)

    xh_d = nc.declare_dram_parameter("xh", [T, 128, 2048], bf16, isOutput=False)
    xl_d = nc.declare_dram_parameter("xl", [T, 128, 2048], bf16, isOutput=False)
    wqh_d = nc.declare_dram_parameter("wqh", [128, C1], bf16, isOutput=False)
    wql_d = nc.declare_dram_parameter("wql", [128, C1], bf16, isOutput=False)
    w2p_d = nc.declare_dram_parameter("w2p", [128, 3, C2], fp32r, isOutput=False)
    w2s_d = nc.declare_dram_parameter("w2s", [C1, 3, C2], fp32r, isOutput=False)
    thrb1_d = nc.declare_dram_parameter("thrb1", [C1, 4], fp32, isOutput=False)
    thrb2_d = nc.declare_dram_parameter("thrb2", [C2, 4], fp32, isOutput=False)
    wh_d = nc.declare_dram_parameter("whm", [C2, OUT], fp32, isOutput=False)

    lgt_d = nc.declare_dram_parameter("lgt", [T, OUT], fp32, isOutput=True)
    s1s_d = nc.declare_dram_parameter("s1s", [C1, T], fp32, isOutput=True)
    s2s_d = nc.declare_dram_parameter("s2s", [C2, T], fp32, isOutput=True)

    with tile.TileContext(nc) as tc:
        with tc.tile_pool(name="consts", bufs=1) as consts, \
             tc.tile_pool(name="xin", bufs=2) as xin, \
             tc.tile_pool(name="work", bufs=4) as work, \
             tc.tile_pool(name="p1p", bufs=4, space="PSUM") as p1p, \
             tc.tile_pool(name="p2p", bufs=3, space="PSUM") as p2p, \
             tc.tile_pool(name="php", bufs=1, space="PSUM") as php:

            wqh = consts.tile([128, C1], bf16)
            nc.sync.dma_start(wqh, wqh_d[:])
            wql = consts.tile([128, C1], bf16)
            nc.sync.dma_start(wql, wql_d[:])
            w2p = consts.tile([128, 3, C2], fp32r)
            nc.sync.dma_start(w2p, w2p_d[:])
            w2s = consts.tile([C1, 3, C2], fp32r)
            nc.sync.dma_start(w2s, w2s_d[:])
            thrb1 = consts.tile([C1, 4], fp32)
            nc.sync.dma_start(thrb1, thrb1_d[:])
            thrb2 = consts.tile([C2, 4], fp32)
            nc.sync.dma_start(thrb2, thrb2_d[:])
            whm = consts.tile([C2, OUT], fp32)
            nc.sync.dma_start(whm, wh_d[:])
            thr1, bia1 = thrb1[:, 0:1], thrb1[:, 1:2]
            bia2 = thrb2[:, 1:2]
            nthr2 = thrb2[:, 2:3]

            v1 = consts.tile([C1, RB * AB], fp32)
            nc.vector.memset(v1, 0.0)
            v2 = consts.tile([C2, RB * AB], fp32)
            nc.vector.memset(v2, 0.0)
            # spike image: lower 64 partitions = padded image A, upper 64 =
            # A shifted by one flat element (DMA-maintained) for tap pairing
            ss = [consts.tile([128, PH, PW], fp32r, name=f"ss{i}", tag=f"ss{i}")
                  for i in range(2)]
            for i in range(2):
                nc.vector.memset(ss[i].bitcast(fp32), 0.0)
            s1bs = consts.tile([C1, T * NBLK], fp32)
            s2bs = consts.tile([C2, T * NBLK], fp32)
            s1sum = consts.tile([C1, T], fp32)
            s2sum = consts.tile([C2, T], fp32)

            def conv2_lif2(t, b):
                sp = ss[t % 2]
                p2 = p2p.tile([C2, 512], fp32, tag="p2", name=f"p2_{t}_{b}")
                # pair matmuls (K=128): taps (0,1),(4,5),(6,7)
                for j, (ka, _) in enumerate(PAIRS):
                    dy, dx = ka // 3, ka % 3
                    rhs = sp[:, 4 * b + dy: 4 * b + dy + 4, dx: dx + AB]
                    nc.tensor.matmul(p2, w2p[:, j, :], rhs,
                                     start=(j == 0), stop=False)
                # singles (K=64): taps 2,3,8
                for j, k in enumerate(SINGLES):
                    dy, dx = k // 3, k % 3
                    rhs = sp[0:C1, 4 * b + dy: 4 * b + dy + 4, dx: dx + AB]
                    nc.tensor.matmul(p2, w2s[:, j, :], rhs,
                                     start=False, stop=(j == 2))
                bs = slice(b * 512, (b + 1) * 512)
                u2 = work.tile([C2, 512], fp32, tag="u2", name=f"u2_{t}_{b}")
                nc.vector.scalar_tensor_tensor(
                    out=u2, in0=v2[:, bs], scalar=BETA, in1=p2,
                    op0=Alu.mult, op1=Alu.add)
                # spike2 on ScalarE: tmp = sign(u2 - thr2); s2 = relu(tmp)
                tmp2 = work.tile([C2, 512], fp32, tag="tmp2", name=f"tmp2_{t}_{b}")
                nc.scalar.activation(out=tmp2, in_=u2, func=Act.Sign, bias=nthr2)
                s2t = work.tile([C2, 512], bf16, tag="s2t", name=f"s2t_{t}_{b}")
                nc.scalar.activation(
                    out=s2t, in_=tmp2, func=Act.Relu,
                    accum_out=s2bs[:, t * NBLK + b: t * NBLK + b + 1])
                if ZB:
                    nc.gpsimd.tensor_sub(v2[:, bs], u2, s2t)
                else:
                    nc.vector.scalar_tensor_tensor(
                        out=v2[:, bs], in0=u2, scalar=bia2, in1=s2t,
                        op0=Alu.add, op1=Alu.subtract)

            for t in range(T):
                xht = xin.tile([128, 2048], bf16, tag="xht", name=f"xht_{t}")
                nc.sync.dma_start(xht, xh_d[t])
                xlt = xin.tile([128, 2048], bf16, tag="xlt", name=f"xlt_{t}")
                nc.sync.dma_start(xlt, xl_d[t])
                sp = ss[t % 2]
                for q in range(4):
                    qs = slice(512 * q, 512 * (q + 1))
                    ps = [p1p.tile([C1, 512], fp32, tag="p1", name=f"p1_{t}_{q}_{g}")
                          for g in range(4)]
                    for s, (wt, xt) in enumerate(((wqh, xht), (wql, xht), (wqh, xlt))):
                        for g in range(4):
                            nc.tensor.matmul(
                                ps[g], wt[32 * g: 32 * g + 27, :],
                                xt[32 * g: 32 * g + 27, qs],
                                start=(s == 0), stop=(s == 2),
                                tile_position=(32 * g, 0))
                    for g in range(4):
                        b = 4 * q + g
                        bs = slice(b * 512, (b + 1) * 512)
                        u1 = work.tile([C1, 512], fp32, tag="u1", name=f"u1_{t}_{b}")
                        nc.vector.scalar_tensor_tensor(
                            out=u1, in0=v1[:, bs], scalar=BETA, in1=ps[g],
                            op0=Alu.mult, op1=Alu.add)
                        spike_view = sp[0:C1, 4 * b + 1: 4 * b + 5, 1: AB + 1]
                        nc.vector.tensor_scalar(
                            out=spike_view,
                            in0=u1.rearrange("p (a x) -> p a x", a=4),
                            scalar1=thr1, scalar2=0.0,
                            op0=Alu.is_gt, op1=Alu.add,
                            accum_out=s1bs[:, t * NBLK + b: t * NBLK + b + 1])
                        if ZB:
                            nc.gpsimd.tensor_sub(
                                v1[:, bs].rearrange("p (a x) -> p a x", a=4),
                                u1.rearrange("p (a x) -> p a x", a=4),
                                spike_view)
                        else:
                            nc.vector.scalar_tensor_tensor(
                                out=v1[:, bs].rearrange("p (a x) -> p a x", a=4),
                                in0=u1.rearrange("p (a x) -> p a x", a=4),
                                scalar=bia1, in1=spike_view,
                                op0=Alu.add, op1=Alu.subtract)
                        # maintain shifted copy in upper partitions
                        nc.sync.dma_start(
                            out=sp[C1:128, 4 * b + 1: 4 * b + 5, 0: PW - 1],
                            in_=sp[0:C1, 4 * b + 1: 4 * b + 5, 1: PW])
                        if b >= 1:
                            conv2_lif2(t, b - 1)
                conv2_lif2(t, NBLK - 1)

            nc.vector.reduce_sum(
                s1sum, s1bs.rearrange("p (t b) -> p t b", t=T),
                axis=mybir.AxisListType.X)
            nc.vector.reduce_sum(
                s2sum, s2bs.rearrange("p (t b) -> p t b", t=T),
                axis=mybir.AxisListType.X)
            nc.sync.dma_start(s1s_d[:], s1sum)
            nc.sync.dma_start(s2s_d[:], s2sum)

            for half in range(2):
                ph = php.tile([T, 500], fp32, tag="ph", name=f"ph_{half}")
                nc.tensor.matmul(ph, s2sum, whm[:, half * 500: (half + 1) * 500],
                                 start=True, stop=True)
                lo = work.tile([T, 500], fp32, tag="lo", name=f"lo_{half}")
                nc.vector.tensor_copy(lo, ph)
                nc.sync.dma_start(lgt_d[:, half * 500: (half + 1) * 500], lo)

    nc.compile()
    return nc


def _get_nc(ZB):
    if ("v2", ZB) not in _compiled:
        _compiled[("v2", ZB)] = _build(ZB)
    return _compiled[("v2", ZB)]


# ---------------------------------------------------------------------------
# entry point
# ---------------------------------------------------------------------------
def kernel(x_seq, w1, b1, w2, b2, wh, bh):
    from concourse.bass_utils import run_bass_kernel_spmd

    x_seq = np.asarray(x_seq, np.float32)
    w1 = np.asarray(w1, np.float32)
    b1 = np.asarray(b1, np.float32)
    w2 = np.asarray(w2, np.float32)
    b2 = np.asarray(b2, np.float32)
    wh = np.asarray(wh, np.float32)
    bh = np.asarray(bh, np.float32)

    lp_seq = _sample_log_polar(x_seq)              # [T,B,RB,AB,C]
    xcols = _pack_im2col(lp_seq)                   # [B,T,128,2048] f32
    xh = xcols.astype(ml_dtypes.bfloat16)
    xl = (xcols - xh.astype(np.float32)).astype(ml_dtypes.bfloat16)

    w1col = w1.reshape(27, C1).astype(np.float32)
    wq = np.zeros((128, C1), np.float32)
    for g in range(4):
        wq[32 * g: 32 * g + 27] = w1col
    wqh = wq.astype(ml_dtypes.bfloat16)
    wql = (wq - wqh.astype(np.float32)).astype(ml_dtypes.bfloat16)

    w2taps = np.ascontiguousarray(w2.reshape(9, C1, C2))   # [k, cin, cout]
    w2p = np.zeros((128, 3, C2), np.float32)
    for j, (ka, kb) in enumerate(PAIRS):
        w2p[0:C1, j, :] = w2taps[ka]
        w2p[C1:128, j, :] = w2taps[kb]
    w2s = np.ascontiguousarray(
        w2taps[SINGLES].transpose(1, 0, 2))        # [cin, 3, cout]

    thrb1 = np.stack([1.0 - b1, b1, -(1.0 - b1), 0 * b1], axis=1).astype(np.float32)
    thrb2 = np.stack([1.0 - b2, b2, -(1.0 - b2), 0 * b2], axis=1).astype(np.float32)

    base = dict(wqh=wqh, wql=wql, w2p=w2p, w2s=w2s,
                thrb1=thrb1, thrb2=thrb2, whm=wh)

    ZB = bool(np.all(b1 == 0) and np.all(b2 == 0))
    nc = _get_nc(ZB)
    in_maps = [dict(base, xh=np.ascontiguousarray(xh[i]),
                    xl=np.ascontiguousarray(xl[i])) for i in range(B)]
    kw = {}
    if TRACE:
        kw = dict(trace=True, tmpdir=TRACE_DIR)
    res = run_bass_kernel_spmd(nc, in_maps, list(range(NCORES)), **kw)
    last_result_info.clear()
    last_result_info.update(exec_time_ns=res.exec_time_ns,
                            mean_exec_time_ns=res.mean_exec_time_ns)

    # ---- host-side assembly ----
    logits_seq = np.empty((T, B, OUT), np.float32)
    s1_counts = np.empty((B, C1, T), np.float32)
    s2_counts = np.empty((B, C2, T), np.float32)
    for i in range(B):
        r = res.results[i]
        logits_seq[:, i, :] = r["lgt"] * np.float32(1.0 / (RB * AB)) + bh
        s1_counts[i] = r["s1s"]
        s2_counts[i] = r["s2s"]

    readout = logits_seq.mean(axis=0)
    sr_seq = np.stack([
        s1_counts.sum(axis=(0, 1)) / np.float32(B * RB * AB * C1),
        s2_counts.sum(axis=(0, 1)) / np.float32(B * RB * AB * C2),
    ], axis=1).astype(np.float32)                  # [T, 2]
    sr = sr_seq.mean(axis=0).astype(np.float32)
    re_seq = lp_seq.mean(axis=(1, 3, 4))           # [T, RB]
    radial_energy = re_seq.mean(axis=0).astype(np.float32)

    return (readout, logits_seq, sr, lp_seq, radial_energy)


# revision 13
# speedup vs baseline: 1.4794x; 1.0270x over previous
"""Trainium2 Bass kernel for the LogPolarFoveatedConvSNN problem.

Data-parallel over batch: 8 cores, one batch item each. Host does the
(constant-grid) log-polar bilinear gather + im2col packing; the device runs
the T=16 recurrent conv/LIF pipeline; host does the cheap final reductions.

Device design (per core, per step):
  conv1: im2col K=27 matmuls, bf16 3-term split (xh*wh + xh*wl + xl*wh),
         4 spatial blocks concurrent via PE row-tiling.
  LIF1:  decay+add (DVE, PSUM src) -> spike is_gt written into a zero-padded
         [64,66,130] image (lower partitions of a 128-part tile) -> reset.
  conv2: 3x3 taps as matmuls accumulating in PSUM, fp32r (spikes exact),
         taps paired via a DMA-maintained column-shifted copy in upper
         partitions: 3 pair-matmuls (K=128) + 3 singles (K=64).
  LIF2:  decay+add (DVE) -> spike via Sign+Relu on ScalarE (with channel-sum
         accumulation) -> reset (DVE).
Head (once): logits = spike-count matrix @ wh on PE; scaling on host.
"""

import numpy as np
import ml_dtypes

# ---- problem constants (hardcoded per harness contract) ----
T, B, H, W, C = 16, 8, 224, 224, 3
RB, AB = 64, 128
C1, C2, OUT = 64, 128, 1000
BETA, THRESH = 0.9, 1.0
NCORES = 8
NBLK = 16           # spatial blocks per image (each 4 rows x 128 cols = 512)
PW = AB + 2         # padded width 130
PH = RB + 2         # padded height 66

TRACE = False
TRACE_DIR = None

_compiled = {}
last_result_info = {}

# tap index k = 3*dy + dx; flat padded offset = 130*dy + dx
PAIRS = [(0, 1), (4, 5), (6, 7)]     # delta=1 pairs (tap indices)
SINGLES = [2, 3, 8]                  # remaining taps


# ---------------------------------------------------------------------------
# host-side log-polar sampling (mirrors reference._build_log_polar_grid in f32)
# ---------------------------------------------------------------------------
def _grid():
    cy, cx = (H - 1) / 2.0, (W - 1) / 2.0
    max_r = max(min(cy, cx) * 1.0, 1.0 + 0.001)
    radial_steps = np.linspace(0.0, 1.0, RB, dtype=np.float32)
    ang = np.arange(AB, dtype=np.float32) * np.float32(2.0 * np.pi / AB)
    log_min = np.float32(np.log(np.float32(1.0)))
    log_max = np.float32(np.log(np.float32(max_r)))
    radii = np.exp(log_min + radial_steps * (log_max - log_min)).astype(np.float32)
    ys = np.clip(cy + radii[:, None] * np.sin(ang)[None, :], 0.0, H - 1.0).astype(np.float32)
    xs = np.clip(cx + radii[:, None] * np.cos(ang)[None, :], 0.0, W - 1.0).astype(np.float32)
    return ys, xs


def _sample_log_polar(x_seq):
    ys, xs = _grid()
    y0 = np.floor(ys)
    x0 = np.floor(xs)
    wy = (ys - y0)[None, None, :, :, None].astype(np.float32)
    wx = (xs - x0)[None, None, :, :, None].astype(np.float32)
    y0i = np.clip(y0.astype(np.int32), 0, H - 1)
    x0i = np.clip(x0.astype(np.int32), 0, W - 1)
    y1i = np.clip(y0i + 1, 0, H - 1)
    x1i = np.clip(x0i + 1, 0, W - 1)
    g = lambda yy, xx: x_seq[:, :, yy, xx, :]  # [T,B,RB,AB,C]
    return (g(y0i, x0i) * (1 - wy) * (1 - wx)
            + g(y0i, x1i) * (1 - wy) * wx
            + g(y1i, x0i) * wy * (1 - wx)
            + g(y1i, x1i) * wy * wx).astype(np.float32)


def _pack_im2col(lp_seq):
    """lp_seq [T,B,RB,AB,C] -> per-core xcol [B][T, 128, 2048] f32.

    Row-tiled layout: spatial block b = 4q+g covers rows 16q+4g .. +3.
    partition = 32g + (3k + c) for tap k=3dy+dx, channel c.
    free = 512q + 128r + ab for in-block row r.
    """
    xpad = np.zeros((T, B, PH, PW, C), np.float32)
    xpad[:, :, 1:RB + 1, 1:AB + 1, :] = lp_seq
    x9 = np.stack([xpad[:, :, dy:dy + RB, dx:dx + AB, :]
                   for dy in range(3) for dx in range(3)])  # [9,T,B,RB,AB,C]
    x9 = x9.reshape(9, T, B, 4, 4, 4, AB, C)
    x9 = x9.transpose(2, 1, 4, 0, 7, 3, 5, 6)  # [B,t,g,k,c,q,r,ab]
    x9 = x9.reshape(B, T, 4, 27, 4 * 4 * AB)
    out = np.zeros((B, T, 4, 32, 2048), np.float32)
    out[:, :, :, :27, :] = x9
    return out.reshape(B, T, 128, 2048)


# ---------------------------------------------------------------------------
# device kernel build
# ---------------------------------------------------------------------------
def _build(ZB):
    import concourse.mybir as mybir
    import concourse.tile as tile
    from concourse import bacc

    fp32 = mybir.dt.float32
    fp32r = mybir.dt.float32r
    bf16 = mybir.dt.bfloat16
    Alu = mybir.AluOpType
    Act = mybir.ActivationFunctionType

    nc = bacc.Bacc(None, target_bir_lowering=False)

    xh_d = nc.declare_dram_parameter("xh", [T, 128, 2048], bf16, isOutput=False)
    xl_d = nc.declare_dram_parameter("xl", [T, 128, 2048], bf16, isOutput=False)
    wqh_d = nc.declare_dram_parameter("wqh", [128, C1], bf16, isOutput=False)
    wql_d = nc.declare_dram_parameter("wql", [128, C1], bf16, isOutput=False)
    w2p_d = nc.declare_dram_parameter("w2p", [128, 3, C2], fp32r, isOutput=False)
    w2s_d = nc.declare_dram_parameter("w2s", [C1, 3, C2], fp32r, isOutput=False)
    thrb1_d = nc.declare_dram_parameter("thrb1", [C1, 4], fp32, isOutput=False)
    thrb2_d = nc.declare_dram_parameter("thrb2", [C2, 4], fp32, isOutput=False)
    wh_d = nc.declare_dram_parameter("whm", [C2, OUT], fp32, isOutput=False)

    lgt_d = nc.declare_dram_parameter("lgt", [T, OUT], fp32, isOutput=True)
    s1s_d = nc.declare_dram_parameter("s1s", [C1, T], fp32, isOutput=True)
    s2s_d = nc.declare_dram_parameter("s2s", [C2, T], fp32, isOutput=True)

    with tile.TileContext(nc) as tc:
        with tc.tile_pool(name="consts", bufs=1) as consts, \
             tc.tile_pool(name="xin", bufs=3) as xin, \
             tc.tile_pool(name="work", bufs=3) as work, \
             tc.tile_pool(name="p1p", bufs=4, space="PSUM") as p1p, \
             tc.tile_pool(name="p2p", bufs=2, space="PSUM") as p2p:

            wqh = consts.tile([128, C1], bf16)
            nc.sync.dma_start(wqh, wqh_d[:])
            wql = consts.tile([128, C1], bf16)
            nc.sync.dma_start(wql, wql_d[:])
            w2p = consts.tile([128, 3, C2], fp32r)
            nc.sync.dma_start(w2p, w2p_d[:])
            w2s = consts.tile([C1, 3, C2], fp32r)
            nc.sync.dma_start(w2s, w2s_d[:])
            thrb1 = consts.tile([C1, 4], fp32)
            nc.sync.dma_start(thrb1, thrb1_d[:])
            thrb2 = consts.tile([C2, 4], fp32)
            nc.sync.dma_start(thrb2, thrb2_d[:])
            whm = consts.tile([C2, OUT], fp32)
            nc.sync.dma_start(whm, wh_d[:])
            thr1, bia1 = thrb1[:, 0:1], thrb1[:, 1:2]
            bia2 = thrb2[:, 1:2]
            nthr2 = thrb2[:, 2:3]

            v1 = consts.tile([C1, RB * AB], fp32)
            nc.vector.memset(v1, 0.0)
            v2 = consts.tile([C2, RB * AB], fp32)
            nc.vector.memset(v2, 0.0)
            # spike image: lower 64 partitions = padded image A, upper 64 =
            # A shifted by one flat element (DMA-maintained) for tap pairing
            ss = [consts.tile([128, PH, PW], fp32r, name=f"ss{i}", tag=f"ss{i}")
                  for i in range(2)]
            for i in range(2):
                nc.vector.memset(ss[i].bitcast(fp32), 0.0)
            s1bs = consts.tile([C1, T * NBLK], fp32)
            s2bs = consts.tile([C2, T * NBLK], fp32)
            s1sum = consts.tile([C1, T], fp32)
            s2sum = consts.tile([C2, T], fp32)

            def conv2_lif2(t, b0):
                # processes block pair (b0, b0+1); psum [128,1024] spans 2 banks
                sp = ss[t % 2]
                p2 = p2p.tile([C2, 1024], fp32, tag="p2", name=f"p2_{t}_{b0}")
                for i2 in range(2):
                    b = b0 + i2
                    pslice = p2[:, i2 * 512: (i2 + 1) * 512]
                    for j, (ka, _) in enumerate(PAIRS):
                        dy, dx = ka // 3, ka % 3
                        rhs = sp[:, 4 * b + dy: 4 * b + dy + 4, dx: dx + AB]
                        nc.tensor.matmul(pslice, w2p[:, j, :], rhs,
                                         start=(j == 0), stop=False)
                    for j, k in enumerate(SINGLES):
                        dy, dx = k // 3, k % 3
                        rhs = sp[0:C1, 4 * b + dy: 4 * b + dy + 4, dx: dx + AB]
                        nc.tensor.matmul(pslice, w2s[:, j, :], rhs,
                                         start=False, stop=(j == 2))
                bs = slice(b0 * 512, (b0 + 2) * 512)
                u2 = work.tile([C2, 1024], fp32, tag="u2", name=f"u2_{t}_{b0}")
                nc.vector.scalar_tensor_tensor(
                    out=u2, in0=v2[:, bs], scalar=BETA, in1=p2,
                    op0=Alu.mult, op1=Alu.add)
                # spike2 on ScalarE: tmp = sign(u2 - thr2); s2 = relu(tmp)
                tmp2 = work.tile([C2, 1024], bf16, tag="tmp2", name=f"tmp2_{t}_{b0}")
                nc.scalar.activation(out=tmp2, in_=u2, func=Act.Sign, bias=nthr2)
                s2t = work.tile([C2, 1024], bf16, tag="s2t", name=f"s2t_{t}_{b0}")
                nc.scalar.activation(
                    out=s2t[:, 0:512], in_=tmp2[:, 0:512], func=Act.Relu,
                    accum_out=s2bs[:, t * NBLK + b0: t * NBLK + b0 + 1])
                nc.scalar.activation(
                    out=s2t[:, 512:1024], in_=tmp2[:, 512:1024], func=Act.Relu,
                    accum_out=s2bs[:, t * NBLK + b0 + 1: t * NBLK + b0 + 2])
                if ZB:
                    nc.gpsimd.tensor_sub(v2[:, bs], u2, s2t)
                else:
                    nc.vector.scalar_tensor_tensor(
                        out=v2[:, bs], in0=u2, scalar=bia2, in1=s2t,
                        op0=Alu.add, op1=Alu.subtract)

            for t in range(T):
                xht = xin.tile([128, 2048], bf16, tag="xht", name=f"xht_{t}")
                nc.sync.dma_start(xht, xh_d[t])
                xlt = xin.tile([128, 2048], bf16, tag="xlt", name=f"xlt_{t}")
                nc.sync.dma_start(xlt, xl_d[t])
                sp = ss[t % 2]
                for q in range(4):
                    qs = slice(512 * q, 512 * (q + 1))
                    ps = [p1p.tile([C1, 512], fp32, tag="p1", name=f"p1_{t}_{q}_{g}")
                          for g in range(4)]
                    for s, (wt, xt) in enumerate(((wqh, xht), (wql, xht), (wqh, xlt))):
                        for g in range(4):
                            nc.tensor.matmul(
                                ps[g], wt[32 * g: 32 * g + 27, :],
                                xt[32 * g: 32 * g + 27, qs],
                                start=(s == 0), stop=(s == 2),
                                tile_position=(32 * g, 0))
                    for g in range(4):
                        b = 4 * q + g
                        bs = slice(b * 512, (b + 1) * 512)
                        u1 = work.tile([C1, 512], fp32, tag="u1", name=f"u1_{t}_{b}")
                        nc.vector.scalar_tensor_tensor(
                            out=u1, in0=v1[:, bs], scalar=BETA, in1=ps[g],
                            op0=Alu.mult, op1=Alu.add)
                        spike_view = sp[0:C1, 4 * b + 1: 4 * b + 5, 1: AB + 1]
                        nc.vector.tensor_scalar(
                            out=spike_view,
                            in0=u1.rearrange("p (a x) -> p a x", a=4),
                            scalar1=thr1, scalar2=0.0,
                            op0=Alu.is_gt, op1=Alu.add,
                            accum_out=s1bs[:, t * NBLK + b: t * NBLK + b + 1])
                        if ZB:
                            nc.gpsimd.tensor_sub(
                                v1[:, bs].rearrange("p (a x) -> p a x", a=4),
                                u1.rearrange("p (a x) -> p a x", a=4),
                                spike_view)
                        else:
                            nc.vector.scalar_tensor_tensor(
                                out=v1[:, bs].rearrange("p (a x) -> p a x", a=4),
                                in0=u1.rearrange("p (a x) -> p a x", a=4),
                                scalar=bia1, in1=spike_view,
                                op0=Alu.add, op1=Alu.subtract)
                        # maintain shifted copy in upper partitions
                        nc.sync.dma_start(
                            out=sp[C1:128, 4 * b + 1: 4 * b + 5, 0: PW - 1],
                            in_=sp[0:C1, 4 * b + 1: 4 * b + 5, 1: PW])
                        if b >= 2 and b % 2 == 0:
                            conv2_lif2(t, b - 2)
                conv2_lif2(t, NBLK - 2)

            nc.vector.reduce_sum(
                s1sum, s1bs.rearrange("p (t b) -> p t b", t=T),
                axis=mybir.AxisListType.X)
            nc.vector.reduce_sum(
                s2sum, s2bs.rearrange("p (t b) -> p t b", t=T),
                axis=mybir.AxisListType.X)
            nc.sync.dma_start(s1s_d[:], s1sum)
            nc.sync.dma_start(s2s_d[:], s2sum)

            ph = p2p.tile([T, 1000], fp32, tag="p2", name="ph_head")
            for half in range(2):
                nc.tensor.matmul(ph[:, half * 500: (half + 1) * 500], s2sum,
                                 whm[:, half * 500: (half + 1) * 500],
                                 start=True, stop=True)
            lo = work.tile([T, 1000], fp32, tag="lo", name="lo_head")
            nc.vector.tensor_copy(lo, ph)
            nc.sync.dma_start(lgt_d[:], lo)

    nc.compile()
    return nc


def _get_nc(ZB):
    if ("v2", ZB) not in _compiled:
        _compiled[("v2", ZB)] = _build(ZB)
    return _compiled[("v2", ZB)]


# ---------------------------------------------------------------------------
# entry point
# ---------------------------------------------------------------------------
def kernel(x_seq, w1, b1, w2, b2, wh, bh):
    from concourse.bass_utils import run_bass_kernel_spmd

    x_seq = np.asarray(x_seq, np.float32)
    w1 = np.asarray(w1, np.float32)
    b1 = np.asarray(b1, np.float32)
    w2 = np.asarray(w2, np.float32)
    b2 = np.asarray(b2, np.float32)
    wh = np.asarray(wh, np.float32)
    bh = np.asarray(bh, np.float32)

    lp_seq = _sample_log_polar(x_seq)              # [T,B,RB,AB,C]
    xcols = _pack_im2col(lp_seq)                   # [B,T,128,2048] f32
    xh = xcols.astype(ml_dtypes.bfloat16)
    xl = (xcols - xh.astype(np.float32)).astype(ml_dtypes.bfloat16)

    w1col = w1.reshape(27, C1).astype(np.float32)
    wq = np.zeros((128, C1), np.float32)
    for g in range(4):
        wq[32 * g: 32 * g + 27] = w1col
    wqh = wq.astype(ml_dtypes.bfloat16)
    wql = (wq - wqh.astype(np.float32)).astype(ml_dtypes.bfloat16)

    w2taps = np.ascontiguousarray(w2.reshape(9, C1, C2))   # [k, cin, cout]
    w2p = np.zeros((128, 3, C2), np.float32)
    for j, (ka, kb) in enumerate(PAIRS):
        w2p[0:C1, j, :] = w2taps[ka]
        w2p[C1:128, j, :] = w2taps[kb]
    w2s = np.ascontiguousarray(
        w2taps[SINGLES].transpose(1, 0, 2))        # [cin, 3, cout]

    thrb1 = np.stack([1.0 - b1, b1, -(1.0 - b1), 0 * b1], axis=1).astype(np.float32)
    thrb2 = np.stack([1.0 - b2, b2, -(1.0 - b2), 0 * b2], axis=1).astype(np.float32)

    base = dict(wqh=wqh, wql=wql, w2p=w2p, w2s=w2s,
                thrb1=thrb1, thrb2=thrb2, whm=wh)

    ZB = bool(np.all(b1 == 0) and np.all(b2 == 0))
    nc = _get_nc(ZB)
    in_maps = [dict(base, xh=np.ascontiguousarray(xh[i]),
                    xl=np.ascontiguousarray(xl[i])) for i in range(B)]
    kw = {}
    if TRACE:
        kw = dict(trace=True, tmpdir=TRACE_DIR)
    res = run_bass_kernel_spmd(nc, in_maps, list(range(NCORES)), **kw)
    last_result_info.clear()
    last_result_info.update(exec_time_ns=res.exec_time_ns,
                            mean_exec_time_ns=res.mean_exec_time_ns)

    # ---- host-side assembly ----
    logits_seq = np.empty((T, B, OUT), np.float32)
    s1_counts = np.empty((B, C1, T), np.float32)
    s2_counts = np.empty((B, C2, T), np.float32)
    for i in range(B):
        r = res.results[i]
        logits_seq[:, i, :] = r["lgt"] * np.float32(1.0 / (RB * AB)) + bh
        s1_counts[i] = r["s1s"]
        s2_counts[i] = r["s2s"]

    readout = logits_seq.mean(axis=0)
    sr_seq = np.stack([
        s1_counts.sum(axis=(0, 1)) / np.float32(B * RB * AB * C1),
        s2_counts.sum(axis=(0, 1)) / np.float32(B * RB * AB * C2),
    ], axis=1).astype(np.float32)                  # [T, 2]
    sr = sr_seq.mean(axis=0).astype(np.float32)
    re_seq = lp_seq.mean(axis=(1, 3, 4))           # [T, RB]
    radial_energy = re_seq.mean(axis=0).astype(np.float32)

    return (readout, logits_seq, sr, lp_seq, radial_energy)
